# revision 7
# baseline (speedup 1.0000x reference)
"""Trainium2 Bass kernel: single-layer causal attention block (q/k/v/o + RoPE).

Transfer-minimizing sharding: 8 cores, each owns 2 heads x BOTH batches.
Every input/output byte crosses the host<->device link exactly once:
  - core c uploads its x-eighth (rows [512c,512c+512) of the flattened
    [B*S, D] input, a pure view), 2-head row-slices of wq/wk/wv (views),
    its woT row-slice (one host wo.T copy), and a 16-row slice of the
    cos/sin table. x + cos/sin are AllGather'd on-device over NeuronLink.
  - per core compute: q/k/v projections for its 2 heads on both batches
    (weights PE-transposed on device), interleaved RoPE, causal attention
    (64-contraction score matmuls), o_proj partial [B*S, D].
  - partials are ReduceScatter-added over all 8 cores; core c ends with
    exactly rows [512c, 512c+512) of the final output -> host concat.
"""

import os
import sys

import numpy as np

sys.path.insert(0, "/opt/trn_rl_repo")

import concourse.bass as bass  # noqa: E402
import concourse.tile as tile  # noqa: E402
from concourse import bacc, mybir  # noqa: E402
from concourse import bass_utils  # noqa: E402

B, S, D, H, DK = 2, 2048, 1024, 16, 64
NCORES = 8
HPC = H // NCORES  # 2 heads per core
CW = HPC * DK  # 128 head-dim columns per core
VW = DK + 1  # 65: v width per head incl ones column
ND = D // 128  # 8 contraction chunks
NS = S // 128  # 16 s-tiles per batch
NSC = S // 512  # 4 s-chunks per batch
GR = 512 + 16  # gather payload rows per rank: 512 x-rows + 16 csn-rows
ROPE_THETA = 10000.0

F32 = mybir.dt.float32
F32R = mybir.dt.float32r
EXP = mybir.ActivationFunctionType.Exp
ADD = mybir.AluOpType.add
BYPASS = mybir.AluOpType.bypass

ALL8 = [[0, 1, 2, 3, 4, 5, 6, 7]]


def _build_kernel(tc, nc, xq, csnq, wq2, wk2, wv2, wot2, out):
    from contextlib import ExitStack
    _stack = ExitStack()

    # ---- inline constants (travel inside the NEFF, zero per-call cost) ----
    p = np.arange(128)[:, None]
    f = np.arange(512)[None, :]
    mk_np = np.concatenate([(f >= j * 128 + p).astype(np.float32)
                            for j in range(4)], axis=1)
    id_np = np.eye(128, dtype=np.float32)
    ones_np = np.ones((128, HPC), np.float32)
    mkd = nc.inline_tensor(mk_np, name="mkc").ap()
    identd = nc.inline_tensor(id_np, name="identc").ap()
    onesd = nc.inline_tensor(ones_np, name="onesc").ap()

    constp = _stack.enter_context(tc.tile_pool(name="const", bufs=1))
    pers = _stack.enter_context(tc.tile_pool(name="persist", bufs=1))
    dramp = _stack.enter_context(tc.tile_pool(name="dram", bufs=1, space="DRAM"))

    cs_sb = constp.tile([128, S], F32)
    sn_sb = constp.tile([128, S], F32)
    mk_sb = constp.tile([128, 4 * 512], F32)
    id_sb = constp.tile([128, 128], F32R)
    nc.sync.dma_start(mk_sb[:], mkd)
    nc.sync.dma_start(id_sb[:], identd.bitcast(F32R))

    # partition layout for q/k (per head h: [even(32) | odd(32)]):
    #   p = 64*h + 32*t + i  <->  head h, (t=0 even / t=1 odd) dim 2i+t
    q_sb = pers.tile([128, B * S], F32R)
    k_sb = pers.tile([128, B * S], F32R)
    v_sb = pers.tile([128, B * NS * HPC * VW], F32R)
    ctx_sb = pers.tile([128, B * S], F32R)

    gin = dramp.tile([GR, D], F32R)
    gout = dramp.tile([NCORES * GR, D], F32R, addr_space="Shared")
    ob = dramp.tile([B * S, D], F32)
    og = dramp.tile([B * S // NCORES, D], F32)

    # ---- Phase 0: on-device redistribution (AllGather x + cos/sin) ----
    nc.sync.dma_start(gin[0:512, :], xq[:, :])
    nc.sync.dma_start(gin[512:GR, :], csnq[:, :])
    nc.gpsimd.collective_compute(
        "AllGather", BYPASS, replica_groups=ALL8,
        ins=[gin[:].opt()], outs=[gout[:].opt()])

    # cos/sin tables: csn1024 row j lives at gout row GR*(j//16) + 512 + j%16.
    # cs freq-block e (8 rows of width 2048) = csn1024 rows 16e..16e+16
    # (rank e); sn blocks come from ranks 4..7.
    for off in (0, 32, 64, 96):
        for e in range(4):
            src_c = gout[GR * e + 512: GR * e + 528, :].rearrange(
                "(p two) f -> p (two f)", two=2).bitcast(F32)
            nc.sync.dma_start(cs_sb[off + 8 * e: off + 8 * e + 8, :], src_c)
            src_s = gout[GR * (4 + e) + 512: GR * (4 + e) + 528, :].rearrange(
                "(p two) f -> p (two f)", two=2).bitcast(F32)
            nc.sync.dma_start(sn_sb[off + 8 * e: off + 8 * e + 8, :], src_s)

    # ---- Phase 1: weights to SBUF (PE-transposed) + projections + RoPE ----
    with tc.tile_pool(name="wsb", bufs=1) as wsbp, \
         tc.tile_pool(name="tps", bufs=2, space="PSUM") as tps:
        wq_sb = wsbp.tile([128, ND * CW], F32R)
        wk_sb = wsbp.tile([128, ND * CW], F32R)
        wv_sb = wsbp.tile([128, ND * CW], F32R)
        with tc.tile_pool(name="wnat", bufs=1) as wnp:
            wqn = wnp.tile([128, D], F32R)
            wkn = wnp.tile([128, D], F32R)
            wvn = wnp.tile([128, D], F32R)
            # e/o-permuted partition order for q/k: src row 64h+2i+t ->
            # partition 64h+32t+i
            for (wn, w2) in ((wqn, wq2), (wkn, wk2)):
                src4 = w2.rearrange("(h i two) f -> h two i f", h=2, two=2)
                for h in range(2):
                    for t in range(2):
                        nc.sync.dma_start(
                            wn[64 * h + 32 * t: 64 * h + 32 * t + 32, :],
                            src4[h, t])
            nc.sync.dma_start(wvn[:], wv2[:, :])
            for (wn, wsb) in ((wqn, wq_sb), (wkn, wk_sb), (wvn, wv_sb)):
                for d in range(ND):
                    pt = tps.tile([128, 128], F32R, name=f"wt{d}", tag="wt")
                    nc.tensor.transpose(pt[:], wn[:, d * 128:(d + 1) * 128],
                                        id_sb[:])
                    nc.vector.tensor_copy(wsb[:, d * CW:(d + 1) * CW], pt[:])

        with tc.tile_pool(name="xn", bufs=2) as xnp, \
             tc.tile_pool(name="xts", bufs=2) as xtsp, \
             tc.tile_pool(name="pjps", bufs=3, space="PSUM") as pjps, \
             tc.tile_pool(name="vps", bufs=2, space="PSUM") as vps, \
             tc.tile_pool(name="rope", bufs=2) as rtp:
            for b in range(B):
                for sc in range(NSC):
                    e8 = NSC * b + sc
                    xn = xnp.tile([128, 4 * 1024], F32R, name=f"xn{e8}",
                                  tag="xn")
                    for i in range(4):
                        nc.sync.dma_start(
                            xn[:, i * 1024:(i + 1) * 1024],
                            gout[GR * e8 + 128 * i: GR * e8 + 128 * (i + 1), :])
                    xts = xtsp.tile([128, ND * 512], F32R, name=f"xts{e8}",
                                    tag="xts")
                    for i in range(4):
                        for d in range(ND):
                            pt = tps.tile([128, 128], F32R, name=f"xt{i}{d}",
                                          tag="wt")
                            nc.tensor.transpose(
                                pt[:], xn[:, i * 1024 + d * 128:
                                          i * 1024 + (d + 1) * 128], id_sb[:])
                            nc.vector.tensor_copy(
                                xts[:, d * 512 + i * 128: d * 512 + (i + 1) * 128],
                                pt[:])
                    for (dst, wsb) in ((q_sb, wq_sb), (k_sb, wk_sb)):
                        ps = pjps.tile([128, 512], F32, name=f"pj{e8}",
                                       tag="pj")
                        for d in range(ND):
                            nc.tensor.matmul(
                                ps[:], wsb[:, d * CW:(d + 1) * CW],
                                xts[:, d * 512:(d + 1) * 512],
                                start=(d == 0), stop=(d == ND - 1))
                        nc.vector.tensor_copy(
                            dst[:, b * S + sc * 512: b * S + (sc + 1) * 512],
                            ps[:])
                    for i in range(4):
                        pv = vps.tile([128, CW], F32, name=f"pv{i}", tag="pv")
                        for d in range(ND):
                            nc.tensor.matmul(
                                pv[:],
                                xts[:, d * 512 + i * 128: d * 512 + (i + 1) * 128],
                                wv_sb[:, d * CW:(d + 1) * CW],
                                start=(d == 0), stop=(d == ND - 1))
                        sm = b * NS + sc * 4 + i
                        base = sm * HPC * VW
                        dst3 = v_sb[:, base:base + HPC * VW].rearrange(
                            "p (h c) -> p h c", c=VW)
                        nc.vector.tensor_copy(
                            dst3[:, :, 0:DK],
                            pv[:].rearrange("p (h c) -> p h c", c=DK))
                        nc.sync.dma_start(
                            dst3[:, :, DK:DK + 1],
                            onesd.bitcast(F32R).rearrange(
                                "p (h c) -> p h c", c=1))
                # RoPE for this batch (overlaps next batch's projections)
                # re = e*cs - o*sn ; ro = e*sn + o*cs. e lives at partitions
                # [64h,64h+32), o at [64h+32,64h+64). The HW verifier wants
                # TensorTensor SBUF operands on one base partition, so first
                # cross-copy e/o into the opposite half of a temp, then all
                # arithmetic is same-base.
                for t_sb in (q_sb, k_sb):
                    for h in range(2):
                        lo, hi = 64 * h, 64 * h + 32
                        e = t_sb[lo:hi, b * S:(b + 1) * S]
                        o = t_sb[lo + 32:hi + 32, b * S:(b + 1) * S]
                        tmp = rtp.tile([128, S], F32, name="rt", tag="rt")
                        te = tmp[lo + 32:hi + 32, :]  # copy of e (odd base)
                        to = tmp[lo:hi, :]            # copy of o (even base)
                        nc.vector.tensor_copy(te, e)
                        nc.vector.tensor_copy(to, o)
                        nc.vector.tensor_mul(e, e, cs_sb[lo:hi, :])
                        nc.vector.tensor_mul(to, to, sn_sb[lo:hi, :])
                        nc.vector.tensor_sub(e, e, to)
                        nc.vector.tensor_mul(o, o, cs_sb[lo + 32:hi + 32, :])
                        nc.vector.tensor_mul(te, te, sn_sb[lo + 32:hi + 32, :])
                        nc.vector.tensor_add(o, o, te)

    # ---- Phase 3: attention per (batch, head) ----
    with tc.tile_pool(name="sps", bufs=4, space="PSUM") as sps, \
         tc.tile_pool(name="cps", bufs=2, space="PSUM") as cps, \
         tc.tile_pool(name="expool", bufs=6) as exp_pool, \
         tc.tile_pool(name="smp", bufs=4) as smp:
        for b in range(B):
            for h in range(2):
                combo = 2 * b + h
                tp = (64 * (combo % 2), 0)
                for c in range(NSC):
                    nsk = 4 * (c + 1)
                    pctx = cps.tile([VW, 512], F32, name=f"pc{combo}{c}",
                                    tag="pc")
                    exps = []
                    DEPTH = 3

                    def pv_mm(t, nsk=nsk, pctx=pctx, exps=exps, b=b, h=h):
                        vbase = (b * NS + t) * HPC * VW + h * VW
                        nc.tensor.matmul(
                            pctx[:],
                            v_sb[:, vbase:vbase + VW],
                            exps[t][:],
                            start=(t == 0), stop=(t == nsk - 1),
                            skip_group_check=True)

                    for t in range(nsk):
                        pscore = sps.tile([128, 512], F32, name=f"sc{t}",
                                          tag="sc")
                        nc.tensor.matmul(
                            pscore[:],
                            k_sb[64 * h: 64 * h + 64,
                                 b * S + t * 128: b * S + (t + 1) * 128],
                            q_sb[64 * h: 64 * h + 64,
                                 b * S + c * 512: b * S + (c + 1) * 512],
                            start=True, stop=True, tile_position=tp,
                            skip_group_check=True)
                        et = exp_pool.tile([128, 512], F32R, name=f"et{t}",
                                           tag="et")
                        nc.scalar.activation(et[:], pscore[:], EXP, scale=0.125)
                        j = t - 4 * c
                        if j >= 0:
                            nc.vector.tensor_mul(et[:], et[:],
                                                 mk_sb[:, j * 512:(j + 1) * 512])
                        exps.append(et)
                        if t >= DEPTH:
                            pv_mm(t - DEPTH)
                    for t in range(max(0, nsk - DEPTH), nsk):
                        pv_mm(t)
                    rc = smp.tile([1, 512], F32, name="rc", tag="rc")
                    nc.vector.reciprocal(rc[:], pctx[DK:DK + 1, :])
                    rb = smp.tile([64, 512], F32, name="rb", tag="rb")
                    nc.gpsimd.partition_broadcast(rb[:], rc[:])
                    dst = ctx_sb[64 * h: 64 * h + 64,
                                 b * S + c * 512: b * S + (c + 1) * 512]
                    nc.vector.tensor_mul(dst, pctx[0:DK, :], rb[:])

    # ---- Phase 4: o_proj partials -> DRAM ----
    with tc.tile_pool(name="wop", bufs=1) as wop, \
         tc.tile_pool(name="ops", bufs=4, space="PSUM") as opsp, \
         tc.tile_pool(name="obuf", bufs=4) as obp:
        wo_sb = wop.tile([128, D], F32R)
        nc.sync.dma_start(wo_sb[:], wot2[:, :])
        for b in range(B):
            for sm in range(NS):
                for do_ in range(2):
                    po = opsp.tile([128, 512], F32, name=f"po{do_}", tag="po")
                    nc.tensor.matmul(
                        po[:],
                        ctx_sb[:, b * S + sm * 128: b * S + sm * 128 + 128],
                        wo_sb[:, do_ * 512:(do_ + 1) * 512],
                        start=True, stop=True)
                    ot = obp.tile([128, 512], F32, name=f"ot{do_}", tag="ot")
                    nc.scalar.copy(ot[:], po[:])
                    nc.sync.dma_start(
                        ob[b * S + sm * 128: b * S + sm * 128 + 128,
                           do_ * 512:(do_ + 1) * 512], ot[:])

    # ---- Phase 5: ReduceScatter partials; core c keeps output rows
    # [512c, 512c+512) of the flattened [B*S, D] result ----
    nc.gpsimd.collective_compute(
        "ReduceScatter", ADD, replica_groups=ALL8,
        ins=[ob[:].opt()], outs=[og[:].opt()])
    nc.sync.dma_start(out[:, :], og[:, :])
    _stack.close()


def build_nc():
    nc = bacc.Bacc("TRN2", target_bir_lowering=False, debug=False,
                   enable_asserts=False, num_devices=NCORES)
    xq = nc.dram_tensor("xq", [512, D], F32R, kind="ExternalInput").ap()
    csnq = nc.dram_tensor("csnq", [16, D], F32R, kind="ExternalInput").ap()
    wq2 = nc.dram_tensor("wq2", [CW, D], F32R, kind="ExternalInput").ap()
    wk2 = nc.dram_tensor("wk2", [CW, D], F32R, kind="ExternalInput").ap()
    wv2 = nc.dram_tensor("wv2", [CW, D], F32R, kind="ExternalInput").ap()
    wot2 = nc.dram_tensor("wot2", [CW, D], F32R, kind="ExternalInput").ap()
    out = nc.dram_tensor("out", [B * S // NCORES, D], F32,
                         kind="ExternalOutput").ap()
    with tile.TileContext(nc) as tc:
        _build_kernel(tc, nc, xq, csnq, wq2, wk2, wv2, wot2, out)
    nc.compile()
    return nc


def make_in_maps(in_features, q_proj_weight, k_proj_weight, v_proj_weight,
                 o_proj_weight, token_positions):
    x = np.ascontiguousarray(np.asarray(in_features, dtype=np.float32)).reshape(
        B * S, D)
    wq = np.asarray(q_proj_weight, dtype=np.float32)
    wk = np.asarray(k_proj_weight, dtype=np.float32)
    wv = np.asarray(v_proj_weight, dtype=np.float32)
    woT = np.ascontiguousarray(np.asarray(o_proj_weight, dtype=np.float32).T)
    pos = np.asarray(token_positions).astype(np.float64)

    inv = ROPE_THETA ** (-2.0 * np.arange(DK // 2, dtype=np.float64) / DK)
    ang = inv[:, None] * pos[None, :]  # [32, S]
    csn = np.concatenate([np.cos(ang), np.sin(ang)]).astype(np.float32)
    csn1024 = np.ascontiguousarray(csn).reshape(128, 1024)

    in_maps = []
    for c in range(NCORES):
        in_maps.append({
            "xq": x[512 * c: 512 * (c + 1)],
            "csnq": csn1024[16 * c: 16 * (c + 1)],
            "wq2": wq[CW * c: CW * (c + 1)],
            "wk2": wk[CW * c: CW * (c + 1)],
            "wv2": wv[CW * c: CW * (c + 1)],
            "wot2": woT[CW * c: CW * (c + 1)],
        })
    return in_maps


_NC_CACHE = []
last_exec_ns = None


def kernel(in_features, q_proj_weight, k_proj_weight, v_proj_weight,
           o_proj_weight, token_positions, d_model=1024, num_heads=16,
           **_ignored):
    global last_exec_ns
    assert int(d_model) == D and int(num_heads) == H
    in_maps = make_in_maps(in_features, q_proj_weight, k_proj_weight,
                           v_proj_weight, o_proj_weight, token_positions)
    if not _NC_CACHE:
        _NC_CACHE.append(build_nc())
    nc = _NC_CACHE[0]
    trace = bool(int(os.environ.get("KERNEL_TRACE", "0")))
    res = bass_utils.run_bass_kernel_spmd(nc, in_maps,
                                          core_ids=list(range(NCORES)),
                                          trace=trace)
    last_exec_ns = res.exec_time_ns
    parts = [r["out"].astype(np.float32) for r in res.results]
    return np.concatenate(parts, axis=0).reshape(B, S, D)
